# revision 5
# baseline (speedup 1.0000x reference)
"""Trainium2 Bass kernel for the maxtext-style quantized KV-cache update.

Computation (see problem reference):
  1. quantize the new decode-step K/V (per-(b,h) abs-max over D, rint)
  2. scatter-append at ar_cache_index into the stored (S,H,B,D) int8-valued
     cache + per-row scales
  3. return the fully dequantized caches  q * scale / 127.5  for K and V.

Strategy: tensor-parallel over heads — 16 heads -> 2 per NeuronCore, 8 cores.
The cache holds int8-valued floats (rint of randn*40, |q| < 2048), which are
exactly representable in fp16 — the host converts the cache to fp16
(lossless) and the device streams fp16 in and fp16 out, halving HBM traffic
versus f32.  The fp16 output (~5e-4 relative error) is upcast on the host.

Layout: each core's 49,152 cache rows (K then V, row = one (s,h,b) D-vector,
64 rows per SBUF partition) are stored d-major *within* each partition:
element j of a partition = (d, c) = (j // 64, j % 64) of its 64-row slab.
The dequant multiply is then ct[p, d, c] *= scale16[p, c] — a broadcast
along the *middle* axis, so every DVE operand keeps a packed 2-byte last
dim and the multiply runs in the 2x_1p fast path (~4.3us per 2 MiB tile),
staying off the DMA-bound critical path.  Scales are pre-multiplied by
1/127.5 and pre-cast to fp16 on the host.
"""

import os
import sys

if "/opt/trn_rl_repo" not in sys.path:
    sys.path.insert(0, "/opt/trn_rl_repo")

# The kernel executes through the axon/neuron PJRT backend; a leftover
# JAX_PLATFORMS=cpu (used for reference-side jax) would hide the NeuronCores.
if "jax" not in sys.modules:
    _jp = os.environ.get("JAX_PLATFORMS")
    if _jp is not None and "axon" not in _jp and "neuron" not in _jp:
        del os.environ["JAX_PLATFORMS"]

import numpy as np

B, H, D = 4, 16, 128
S_AR = 3072
NCORES = 8
HSH = H // NCORES            # heads per core
ROWS = S_AR * HSH * B        # rows per core-cache (24576)
F = 8192                     # SBUF tile free dim (elements)
CPS = F // D                 # rows (columns) per partition slab (64)
NT = 2 * ROWS * D // (128 * F)   # tiles over combined K+V rows (6)
TPC = NT // 2                # tiles per cache (3)
C_DEQ = float(np.float32(1.0 / 127.5))
MAX_INT8 = 127.5
MAGIC = 12582912.0           # 1.5 * 2**23: (x + MAGIC) - MAGIC == rint(x) in f32
NCHUNK = 4                   # free-dim chunks for the final (patch-free) tile

TRACE = False                # test harness sets True to capture an NTFF profile
LAST_RESULT = None           # BassKernelResults of the most recent run

_PROG_CACHE = {}


def _build_program(s: int):
    import concourse.bacc as bacc
    import concourse.mybir as mybir
    from concourse.tile import TileContext

    f32 = mybir.dt.float32
    f16 = mybir.dt.float16
    op = mybir.AluOpType

    nc = bacc.Bacc("TRN2", target_bir_lowering=False, debug=False,
                   num_devices=NCORES)

    cin = nc.dram_tensor("cin", [NT, 128, F], f16, kind="ExternalInput")
    sc = nc.dram_tensor("sc", [NT, 128, CPS], f16, kind="ExternalInput")
    nk = nc.dram_tensor("nk", [1, HSH * B * D], f32, kind="ExternalInput")
    nv = nc.dram_tensor("nv", [1, HSH * B * D], f32, kind="ExternalInput")
    out = nc.dram_tensor("out", [NT, 128, F], f16, kind="ExternalOutput")

    # patch site of the replacement row for each cache: rows [8s, 8s+8) of
    # the cache's 24576 rows; 64-row slabs -> tile, partition, column
    NR = HSH * B                              # 8 rows per seq position
    patch = {}
    for i, nm in enumerate(("k", "v")):
        slab = i * (ROWS // CPS) + (s * NR) // CPS
        t_star, p_star = divmod(slab, 128)
        c0 = (s * NR) % CPS
        patch.setdefault(t_star, []).append((nm, p_star, c0))
    order = sorted(range(NT), key=lambda t: (t not in patch, t))
    assert order[-1] not in patch

    with TileContext(nc) as tc:
        with tc.tile_pool(name="row", bufs=1) as rowpool, \
             tc.tile_pool(name="cp", bufs=4) as cpool, \
             tc.tile_pool(name="sp", bufs=4) as spool:
            # --- dequantized replacement row, d-major on one partition ---
            # nk/nv arrive d-major: element j = (d, r) = (j // 8, j % 8).
            drow = {}
            for nm, nt_in in (("k", nk), ("v", nv)):
                rt = rowpool.tile([1, NR * D], f32, tag=f"rt_{nm}")
                nc.gpsimd.dma_start(rt[:], nt_in[:])
                rt3 = rt[:].rearrange("p (d r) -> p d r", r=NR)
                # per-row absmax: view rows as the outer free axis (strided)
                rt3_t = rt[:].rearrange("p (d r) -> p r d", r=NR)
                sig = rowpool.tile([1, NR], f32, tag=f"sig_{nm}")
                nc.vector.tensor_reduce(sig[:].unsqueeze(2), rt3_t,
                                        axis=mybir.AxisListType.X,
                                        op=op.max, apply_absolute_value=True)
                rc = rowpool.tile([1, NR], f32, tag=f"rc_{nm}")
                nc.vector.reciprocal(rc[:], sig[:])
                rr = rowpool.tile([1, NR], f32, tag=f"rr_{nm}")
                nc.vector.tensor_scalar(rr[:], rc[:], MAX_INT8, None, op.mult)
                s2 = rowpool.tile([1, NR], f32, tag=f"s2_{nm}")
                nc.vector.tensor_scalar(s2[:], sig[:], C_DEQ, None, op.mult)
                rr_b = rr[:].unsqueeze(1).broadcast_to((1, D, NR))
                s2_b = s2[:].unsqueeze(1).broadcast_to((1, D, NR))
                tt = rowpool.tile([1, NR * D], f32, tag=f"tt_{nm}")
                tt3 = tt[:].rearrange("p (d r) -> p d r", r=NR)
                nc.vector.tensor_tensor(tt3, rt3, rr_b, op.mult)
                qt = rowpool.tile([1, NR * D], f32, tag=f"qt_{nm}")
                nc.vector.tensor_scalar(qt[:], tt[:], MAGIC, None, op.add)
                q2 = rowpool.tile([1, NR * D], f32, tag=f"q2_{nm}")
                nc.vector.tensor_scalar(q2[:], qt[:], MAGIC, None, op.subtract)
                dr = rowpool.tile([1, NR * D], f16, tag=f"dr_{nm}")
                dr3 = dr[:].rearrange("p (d r) -> p d r", r=NR)
                nc.vector.tensor_tensor(dr3, q2[:].rearrange(
                    "p (d r) -> p d r", r=NR), s2_b, op.mult)
                drow[nm] = dr

            # --- bulk dequantize: out = q * scale16, 2x_1p fp16 ---
            for t in order:
                last = t == order[-1]
                ct = cpool.tile([128, F], f16, tag="ct")
                st = spool.tile([128, CPS], f16, tag="st")
                nc.gpsimd.dma_start(st[:], sc[t])
                nchunk = NCHUNK if last else 1
                dper = D // nchunk              # d values per chunk
                for ci in range(nchunk):
                    fsl = slice(ci * (F // nchunk), (ci + 1) * (F // nchunk))
                    dsl = slice(ci * dper, (ci + 1) * dper)
                    nc.sync.dma_start(ct[:, fsl], cin[t, :, fsl])
                    ct3 = ct[:, fsl].rearrange("p (d c) -> p d c", c=CPS)
                    stb = st[:].unsqueeze(1).broadcast_to((128, dper, CPS))
                    nc.vector.tensor_tensor(ct3, ct3, stb, op.mult)
                    for nm, p_star, c0 in (patch.get(t, ()) if ci == nchunk - 1
                                           else ()):
                        tgt = ct[p_star:p_star + 1].rearrange(
                            "p (d c) -> p d c", c=CPS)[:, :, c0:c0 + NR]
                        nc.sync.dma_start(tgt, drow[nm][:].rearrange(
                            "p (d r) -> p d r", r=NR))
                    nc.scalar.dma_start(out[t, :, fsl], ct[:, fsl])
    nc.compile()
    return nc


def _prog(s: int):
    if s not in _PROG_CACHE:
        _PROG_CACHE[s] = _build_program(s)
    return _PROG_CACHE[s]


def _to_dmajor(rows16):
    """(24576, 128) fp16 row-major -> (TPC, 128, F) d-major per 64-row slab."""
    a = rows16.reshape(TPC, 128, CPS, D)      # [t, p, c, d]
    return np.ascontiguousarray(a.transpose(0, 1, 3, 2)).reshape(TPC, 128, F)


def _from_dmajor(tiles16):
    """(TPC, 128, F) fp16 d-major -> (24576, 128) f32 row-major."""
    a = tiles16.reshape(TPC, 128, D, CPS).transpose(0, 1, 3, 2)
    return a.astype(np.float32).reshape(ROWS, D)


def kernel(key, value, cached_ar_key, cached_ar_value,
           cached_ar_key_scale, cached_ar_value_scale, ar_cache_index):
    global LAST_RESULT
    from concourse.bass_utils import run_bass_kernel_spmd

    key = np.asarray(key, dtype=np.float32)
    value = np.asarray(value, dtype=np.float32)
    cached_ar_key = np.asarray(cached_ar_key, dtype=np.float32)
    cached_ar_value = np.asarray(cached_ar_value, dtype=np.float32)
    cached_ar_key_scale = np.asarray(cached_ar_key_scale, dtype=np.float32)
    cached_ar_value_scale = np.asarray(cached_ar_value_scale, dtype=np.float32)
    s = int(ar_cache_index)

    nc = _prog(s)

    # int8-valued cache entries are exact in fp16
    k16 = cached_ar_key.astype(np.float16)
    v16 = cached_ar_value.astype(np.float16)
    key_t = np.ascontiguousarray(key[:, 0].transpose(1, 0, 2))      # (H,B,D)
    val_t = np.ascontiguousarray(value[:, 0].transpose(1, 0, 2))

    in_maps = []
    for i in range(NCORES):
        h0 = i * HSH
        hs = slice(h0, h0 + HSH)
        cin = np.empty((NT, 128, F), np.float16)
        cin[:TPC] = _to_dmajor(k16[:, hs].reshape(ROWS, D))
        cin[TPC:] = _to_dmajor(v16[:, hs].reshape(ROWS, D))
        scf = np.empty((NT, 128, CPS), np.float32)
        scf[:TPC] = cached_ar_key_scale[:, hs].reshape(TPC, 128, CPS)
        scf[TPC:] = cached_ar_value_scale[:, hs].reshape(TPC, 128, CPS)
        in_maps.append({
            "cin": cin,
            "sc": (scf * np.float32(C_DEQ)).astype(np.float16),
            # d-major flat new rows: element j = (d, r)
            "nk": np.ascontiguousarray(
                key_t[hs].reshape(HSH * B, D).T).reshape(1, -1),
            "nv": np.ascontiguousarray(
                val_t[hs].reshape(HSH * B, D).T).reshape(1, -1),
        })

    res = run_bass_kernel_spmd(nc, in_maps, list(range(NCORES)), trace=TRACE)
    LAST_RESULT = res

    k_out = np.empty((S_AR, H, B, D), np.float32)
    v_out = np.empty((S_AR, H, B, D), np.float32)
    for i, r in enumerate(res.results):
        h0 = i * HSH
        o = np.asarray(r["out"])
        k_out[:, h0:h0 + HSH] = _from_dmajor(o[:TPC]).reshape(S_AR, HSH, B, D)
        v_out[:, h0:h0 + HSH] = _from_dmajor(o[TPC:]).reshape(S_AR, HSH, B, D)
    return k_out, v_out
